# revision 1
# baseline (speedup 1.0000x reference)
"""Distributed causal-attention block (dense_transformer) on 8 TRN2 NeuronCores.

Sharding: data-parallel over batch (b=2) x tensor-parallel over head pairs
(8 heads -> 4 groups of 2). Core i handles batch i//4, heads (2*(i%4), 2*(i%4)+1).

Per-core pipeline (interleaved): for each 512-token tile nt, project that
tile's Q^T/K^T/V (transposed layouts so attention is transpose-free), then run
block-causal attention for q-tile nt -- it only needs the K/V prefix <= nt, so
softmax exp (ScalarE, the bottleneck engine) starts ~15us into the kernel and
the projection matmuls act as PE filler, keeping the tensor engine dense/warm.

Softmax denominators come free via an augmented ones-column in V (row 64 of
the PV accumulator); 1/l is computed on a DMA-broadcast [64, 2*QW] copy so the
normalization folds into the PSUM->SBUF copy of the attention output, letting
the two heads' output projections accumulate into a single PSUM bank (one
bias-add per 128-row block instead of two scalar_tensor_tensor passes).

Output reduction: chunked ReduceScatter(add) over each 4-core batch group
(big early chunks amortize the ~15-45us ncfw per-op cost; the last two tiles
are processed in swapped order (7 then 6) so tile 7's RS overlaps tile 6's
~30us of attention and the final RS is a small single-tile chunk). Everything
crossing cores is bf16 end-to-end (the host casts the final output to f32),
halving RS wire bytes; RS-output gathers are single DRAM->DRAM DMAs on the
otherwise-idle GpSimd queue, so no compute engine ever has a collective-
dependent instruction in its in-order stream -- Tile's scheduler otherwise
hoists such waits into the middle of the Vector stream, head-of-line-blocking
the whole pipeline for the ~20-45us an RS takes on this fabric.

Scheduling invariant that matters most: the o0/o1 PSUM accumulator rotation
(and with it the exp stream) depends only on SBUF/PSUM-local copies, never on
a DMA -- ReduceScatter HBM traffic stalls any DMA by ~20us+, and an HBM hop
on the inter-tile critical path serializes the whole kernel behind the
collectives (measured: +35us).

B, S, D, H = 2, 4096, 512, 8 (hd=64). Hardcoded per problem spec.
"""

import numpy as np
import ml_dtypes

import concourse.bacc as bacc
import concourse.mybir as mybir
from concourse import tile
from concourse.bass_utils import run_bass_kernel_spmd

B, S, D = 2, 4096, 512
H = 8
HD = D // H          # 64
NCORES = 8
R = 128              # qkv rows per core (2 heads x 64)
S4 = S // 4          # output rows per core
NT = 8               # q tiles of 512
QW = 512             # q tile width
CHUNKS = [(0, 2), (2, 2), (4, 2), (7, 1), (6, 1)]   # (start tile, n tiles)
CHOFF = [0, 256, 512, 768, 896]                     # per-core out row offsets
# attention processing order: tile 7 BEFORE tile 6, so tile 7's
# ReduceScatter overlaps tile 6's ~30us of attention and the final RS
# is the small tile-6 chunk right at the end of compute
ORDER = [0, 1, 2, 3, 4, 5, 7, 6]
PROJ_FOR = {7: [6, 7], 6: []}    # tile 7's attention needs k/v of tile 6 too

BF16 = mybir.dt.bfloat16
F32 = mybir.dt.float32
AF = mybir.ActivationFunctionType
BF16_NP = ml_dtypes.bfloat16

_CACHE = {}


def _build_nc():
    nc = bacc.Bacc(num_devices=NCORES)

    # host pre-arranged so every weight DMA is contiguous
    xT = nc.declare_dram_parameter("xT", [D, S], BF16, isOutput=False)
    wqL = nc.declare_dram_parameter("wqL", [128, D], BF16, isOutput=False)
    wkL = nc.declare_dram_parameter("wkL", [128, D], BF16, isOutput=False)
    wvL = nc.declare_dram_parameter("wvL", [128, 4 * 130], BF16, isOutput=False)
    bq = nc.declare_dram_parameter("bq", [R, 1], F32, isOutput=False)
    bk = nc.declare_dram_parameter("bk", [R, 1], F32, isOutput=False)
    bvb = nc.declare_dram_parameter("bvb", [128, 130], F32, isOutput=False)
    wo0 = nc.declare_dram_parameter("wo0", [HD, D], BF16, isOutput=False)
    wo1 = nc.declare_dram_parameter("wo1", [HD, D], BF16, isOutput=False)
    bob4 = nc.declare_dram_parameter("bob4", [128, D], F32, isOutput=False)
    maskc = nc.declare_dram_parameter("maskc", [128, 4 * QW], BF16, isOutput=False)
    out_ext = nc.declare_dram_parameter("out", [S4, D], BF16, isOutput=True)

    parts = [nc.dram_tensor(f"part{ci}", [n * QW, D], BF16)
             for ci, (_, n) in enumerate(CHUNKS)]
    ldram = [nc.dram_tensor(f"ldram{t}", [2 * QW], F32) for t in range(NT)]
    rss = [nc.dram_tensor(f"rs{ci}", [n * 128, D], BF16)
           for ci, (_, n) in enumerate(CHUNKS)]
    # tiny scratch RS to absorb the ~35us ncfw init barrier + first-op
    # latency during the idle early window, so the first real RS starts
    # promptly at its trigger
    cc_warm_in = nc.dram_tensor("ccwi", [4, 128], BF16)
    cc_warm_out = nc.dram_tensor("ccwo", [1, 128], BF16)

    tile2chunk = {}
    for ci, (st_, n) in enumerate(CHUNKS):
        for k in range(n):
            tile2chunk[st_ + k] = (ci, k, k == n - 1)

    with tile.TileContext(nc) as tc:
        with (
            tc.tile_pool(name="const", bufs=1) as cpool,
            tc.tile_pool(name="xres", bufs=1) as xpool,
            tc.tile_pool(name="pt", bufs=24) as ppool,
            tc.tile_pool(name="small", bufs=3) as spool,
            tc.tile_pool(name="stage", bufs=8) as stpool,
            tc.tile_pool(name="ps_s", bufs=2, space="PSUM") as ps_s,
            tc.tile_pool(name="ps_o", bufs=1, space="PSUM") as ps_o,
            tc.tile_pool(name="ps_pp", bufs=1, space="PSUM") as ps_pp,
            tc.tile_pool(name="ps_po", bufs=1, space="PSUM") as ps_po,
        ):
            # ---------- weights/constants + x, ordered so the first q-tile's
            # projection (and with it the exp stream) starts as early as
            # possible: q/k weights -> first 512-token x slice -> v weights ->
            # the rest
            bq_sb = cpool.tile([R, 1], F32)
            nc.sync.dma_start(bq_sb[:], bq[:, :])
            bk_sb = cpool.tile([R, 1], F32)
            nc.sync.dma_start(bk_sb[:], bk[:, :])
            wq_sb = cpool.tile([128, D], BF16)
            nc.sync.dma_start(wq_sb[:], wqL[:, :])
            wk_sb = cpool.tile([128, D], BF16)
            nc.sync.dma_start(wk_sb[:], wkL[:, :])

            xt = []
            for c in range(4):
                t_ = xpool.tile([128, S], BF16, tag=f"xt{c}")
                xt.append(t_)
            for c in range(4):
                nc.sync.dma_start(xt[c][:, 0:QW], xT[128 * c:128 * (c + 1), 0:QW])

            wv_sb = cpool.tile([128, 4 * 130], BF16)
            nc.sync.dma_start(wv_sb[:], wvL[:, :])
            bvb_sb = cpool.tile([128, 130], F32)
            nc.sync.dma_start(bvb_sb[:], bvb[:, :])
            mask_sb = cpool.tile([128, 4 * QW], BF16)
            nc.sync.dma_start(mask_sb[:], maskc[:, :])

            for sl in ([slice(QW, 2 * QW)] +
                       [slice(q * (S // 4), (q + 1) * (S // 4)) for q in (1, 2, 3)]):
                for c in range(4):
                    nc.sync.dma_start(xt[c][:, sl], xT[128 * c:128 * (c + 1), sl])

            wo0_sb = cpool.tile([HD, D], BF16)
            nc.sync.dma_start(wo0_sb[:], wo0[:, :])
            wo1_sb = cpool.tile([HD, D], BF16)
            nc.sync.dma_start(wo1_sb[:], wo1[:, :])
            bob4_sb = cpool.tile([128, D], F32)
            nc.sync.dma_start(bob4_sb[:], bob4[:, :])

            # fire the ncfw warm-up collective immediately
            nc.gpsimd.collective_compute(
                "ReduceScatter",
                mybir.AluOpType.add,
                replica_groups=[[0, 1, 2, 3], [4, 5, 6, 7]],
                ins=[cc_warm_in[:, :]],
                outs=[cc_warm_out[:, :]],
            )

            qT = xpool.tile([128, S], BF16, tag="qT")
            kT = xpool.tile([128, S], BF16, tag="kT")
            vaug = xpool.tile([128, 32 * 130], BF16, tag="vaug")

            for t in ORDER:
                # ---- project q/k/v for this 512-token tile (pp: 1 PSUM bank)
                for pt in PROJ_FOR.get(t, [t]):
                    for w_sb, b_sb, dst in ((wq_sb, bq_sb, qT), (wk_sb, bk_sb, kT)):
                        ps = ps_pp.tile([128, QW], F32, tag="pp")
                        for c in range(4):
                            nc.tensor.matmul(
                                ps[:],
                                w_sb[:, 128 * c:128 * (c + 1)],
                                xt[c][:, QW * pt:QW * (pt + 1)],
                                start=(c == 0), stop=(c == 3),
                            )
                        nc.vector.tensor_scalar_add(
                            dst[:, QW * pt:QW * (pt + 1)], ps[:], b_sb[:])
                    for tb in range(4 * pt, 4 * pt + 4):
                        ps = ps_pp.tile([128, QW], F32, tag="pp")
                        for c in range(4):
                            nc.tensor.matmul(
                                ps[:, 0:130],
                                xt[c][:, 128 * tb:128 * (tb + 1)],
                                wv_sb[:, 130 * c:130 * (c + 1)],
                                start=(c == 0), stop=(c == 3),
                            )
                        nc.vector.tensor_add(
                            vaug[:, 130 * tb:130 * (tb + 1)], ps[:, 0:130],
                            bvb_sb[:]
                        )

                # ---- block-causal attention for q tile t (k blocks 0..4t+3)
                nj = 4 * t + 4
                o0 = ps_o.tile([65, QW], F32, tag="o0")
                o1 = ps_o.tile([65, QW], F32, tag="o1")
                for j in range(nj):
                    # causal: q columns < q0 are fully masked for this k block
                    q0 = max(0, 128 * (j - 4 * t))
                    s = ps_s.tile([128, 2 * QW], F32, tag="s")
                    for h in (0, 1):
                        nc.tensor.matmul(
                            s[:, QW * h + q0:QW * (h + 1)],
                            kT[64 * h:64 * (h + 1), 128 * j:128 * (j + 1)],
                            qT[64 * h:64 * (h + 1), QW * t + q0:QW * (t + 1)],
                            start=True, stop=True,
                        )
                    p = ppool.tile([128, 2 * QW], BF16, tag="p")
                    if q0 == 0:
                        nc.scalar.activation(p[:], s[:], AF.Exp, bias=0.0, scale=0.125)
                    else:
                        sv = s[:].rearrange("k (h q) -> k h q", h=2)[:, :, q0:QW]
                        pv = p[:].rearrange("k (h q) -> k h q", h=2)[:, :, q0:QW]
                        nc.scalar.activation(pv, sv, AF.Exp, bias=0.0, scale=0.125)
                    if j >= 4 * t:  # diagonal 128-col boundary: 0/1 mask (r=0 tile)
                        for h in (0, 1):
                            nc.vector.tensor_mul(
                                p[:, QW * h + q0:QW * h + q0 + 128],
                                p[:, QW * h + q0:QW * h + q0 + 128],
                                mask_sb[:, 0:128],
                            )
                    for h, oo in ((0, o0), (1, o1)):
                        nc.tensor.matmul(
                            oo[:, q0:QW],
                            vaug[:, 130 * j + 65 * h:130 * j + 65 * (h + 1)],
                            p[:, QW * h + q0:QW * (h + 1)],
                            start=(j == 0), stop=(j == nj - 1),
                        )

                # ---- release the o accumulators with SBUF/PSUM-only copies:
                # the o0/o1 bank rotation (and with it the whole attention /
                # exp stream) must NEVER wait on a DMA -- ReduceScatter HBM
                # traffic can stall any DMA by ~20us+.
                oc0 = spool.tile([HD, QW], BF16, tag="oc0")
                oc1 = spool.tile([HD, QW], BF16, tag="oc1")
                nc.vector.tensor_copy(oc0[:], o0[0:64, :])
                nc.vector.tensor_copy(oc1[:], o1[0:64, :])
                lrow = spool.tile([128, 2 * QW], F32, tag="lrow")
                nc.vector.tensor_copy(lrow[64:65, 0:QW], o0[64:65, :])
                nc.vector.tensor_copy(lrow[64:65, QW:2 * QW], o1[64:65, :])

                # denominators: l row -> DRAM -> broadcast-read into 64
                # partitions -> 64-lane fast reciprocal (base-0). This DMA
                # round trip gates only the O-projection (which has stage-pool
                # slack), not the attention stream.
                nc.sync.dma_start(ldram[t][:].rearrange("(a q) -> a q", a=1),
                                  lrow[64:65, :])
                lbc = spool.tile([HD, 2 * QW], F32, tag="lbc")
                nc.sync.dma_start(lbc[:, :], ldram[t][:].partition_broadcast(HD))
                linv = spool.tile([HD, 2 * QW], F32, tag="linv")
                nc.vector.reciprocal_approx_fast(linv[:], lbc[:])

                # normalized attention output so both heads' O-projections can
                # accumulate into a single PSUM bank
                ocn0 = spool.tile([HD, QW], BF16, tag="ocn0")
                ocn1 = spool.tile([HD, QW], BF16, tag="ocn1")
                nc.vector.tensor_mul(ocn0[:], oc0[:], linv[:, 0:QW])
                nc.vector.tensor_mul(ocn1[:], oc1[:], linv[:, QW:2 * QW])

                ci, k, last_in_chunk = tile2chunk[t]
                for tb in range(4):
                    po = ps_po.tile([128, QW], F32, tag="po")
                    nc.tensor.matmul(po[:], ocn0[:, 128 * tb:128 * (tb + 1)],
                                     wo0_sb[:], start=True, stop=False)
                    nc.tensor.matmul(po[:], ocn1[:, 128 * tb:128 * (tb + 1)],
                                     wo1_sb[:], start=False, stop=True)
                    st = stpool.tile([128, QW], BF16, tag="st")
                    nc.vector.tensor_add(st[:], po[:], bob4_sb[:])
                    # parts writes ride the Sync queue: the GpSimd queue's
                    # first collective trigger blocks until the ~35us ncfw
                    # init barrier completes, and parts must not sit behind it
                    nc.sync.dma_start(
                        parts[ci][QW * k + 128 * tb:QW * k + 128 * (tb + 1), :],
                        st[:],
                    )
                if last_in_chunk:
                    # chunked ReduceScatter: overlaps with later q tiles
                    nc.gpsimd.collective_compute(
                        "ReduceScatter",
                        mybir.AluOpType.add,
                        replica_groups=[[0, 1, 2, 3], [4, 5, 6, 7]],
                        ins=[parts[ci][:, :]],
                        outs=[rss[ci][:, :]],
                    )

            # ---- gather RS outputs: f32 end-to-end, one DRAM->DRAM DMA per
            # chunk on the idle GpSimd queue -- no Vector cast / SBUF bounce,
            # so a DMA waiting on a slow RS can never block a compute engine
            for ci, (st_, n) in enumerate(CHUNKS):
                nc.gpsimd.dma_start(
                    out_ext[CHOFF[ci]:CHOFF[ci] + n * 128, :], rss[ci][:, :])

    nc.finalize()
    return nc


def _make_in_maps(x, Wqkv, bqkv, Wo, bo):
    # causal 0/1 multiplicative masks for the 4 diagonal sub-block offsets:
    # keep (p, o) where o >= 128*r + p  (k = 128*(4t+r)+p, q = 512*t+o)
    p_idx = np.arange(128)[:, None]
    o_idx = np.arange(QW)[None, :]
    maskc = np.concatenate(
        [(o_idx >= 128 * r + p_idx).astype(np.float32) for r in range(4)], axis=1
    ).astype(BF16_NP)

    in_maps = []
    for core in range(NCORES):
        b = core // 4
        g = core % 4
        rows = slice(128 * g, 128 * (g + 1))
        wq = Wqkv[0:D][rows]            # [128, 512]
        wk = Wqkv[D:2 * D][rows]
        wv = Wqkv[2 * D:3 * D][rows]
        wvT = np.zeros((D, 130), dtype=np.float32)
        wvT[:, 0:64] = wv[0:64].T
        wvT[:, 65:129] = wv[64:128].T
        bvb = np.zeros((128, 130), dtype=np.float32)
        bvb[:, 0:64] = bqkv[2 * D:3 * D][rows][0:64][None, :]
        bvb[:, 64] = 1.0
        bvb[:, 65:129] = bqkv[2 * D:3 * D][rows][64:128][None, :]
        bvb[:, 129] = 1.0
        # device SBUF layouts, pre-arranged so DMAs are contiguous:
        # wq_sb[p, 128c+m] = wq.T[128c+p, m]
        wqT = np.ascontiguousarray(wq.T)        # [512, 128]
        wkT = np.ascontiguousarray(wk.T)
        wqL = wqT.reshape(4, 128, 128).transpose(1, 0, 2).reshape(128, 512)
        wkL = wkT.reshape(4, 128, 128).transpose(1, 0, 2).reshape(128, 512)
        wvL = wvT.reshape(4, 128, 130).transpose(1, 0, 2).reshape(128, 520)
        in_maps.append({
            "xT": np.ascontiguousarray(x[b].T).astype(BF16_NP),
            "wqL": np.ascontiguousarray(wqL).astype(BF16_NP),
            "wkL": np.ascontiguousarray(wkL).astype(BF16_NP),
            "wvL": np.ascontiguousarray(wvL).astype(BF16_NP),
            "bq": np.ascontiguousarray(bqkv[0:D][rows][:, None]).astype(np.float32),
            "bk": np.ascontiguousarray(bqkv[D:2 * D][rows][:, None]).astype(np.float32),
            "bvb": bvb,
            "wo0": np.ascontiguousarray(Wo[:, 128 * g:128 * g + 64].T).astype(BF16_NP),
            "wo1": np.ascontiguousarray(Wo[:, 128 * g + 64:128 * (g + 1)].T).astype(BF16_NP),
            "bob4": np.tile((bo / 4.0).astype(np.float32)[None, :], (128, 1)),
            "maskc": maskc,
        })
    return in_maps


def run(x, Wqkv, bqkv, Wo, bo, trace=False):
    if "nc" not in _CACHE:
        _CACHE["nc"] = _build_nc()
    nc = _CACHE["nc"]
    in_maps = _make_in_maps(x, Wqkv, bqkv, Wo, bo)
    res = run_bass_kernel_spmd(nc, in_maps, core_ids=list(range(NCORES)), trace=trace)
    out = np.empty((B, S, D), dtype=np.float32)
    for core in range(NCORES):
        b = core // 4
        r = core % 4
        o = np.asarray(res.results[core]["out"], dtype=np.float32)
        # chunk ci covers q rows [512*st, 512*(st+n)); rank r holds the r-th
        # quarter of the chunk, stored at out rows CHOFF[ci]..+n*128
        for ci, (st_, n) in enumerate(CHUNKS):
            sz = n * 128
            out[b, QW * st_ + sz * r:QW * st_ + sz * (r + 1), :] = \
                o[CHOFF[ci]:CHOFF[ci] + sz]
    return out, res


def kernel(x, Wqkv, bqkv, Wo, bo):
    out, _ = run(np.asarray(x, dtype=np.float32), np.asarray(Wqkv, dtype=np.float32),
                 np.asarray(bqkv, dtype=np.float32), np.asarray(Wo, dtype=np.float32),
                 np.asarray(bo, dtype=np.float32))
    return out



# revision 5
# speedup vs baseline: 1.1066x; 1.1066x over previous
"""Distributed causal-attention block (dense_transformer) on 8 TRN2 NeuronCores.

Sharding: data-parallel over batch (b=2) x tensor-parallel over head pairs
(8 heads -> 4 groups of 2). Core i handles batch i//4, heads (2*(i%4), 2*(i%4)+1).

Per-core pipeline, software-pipelined across q-tiles: the attention j-loop for
tile t interleaves (a) PV one block BEHIND scores, so the PV matmul never sits
at the head of the in-order PE queue waiting for the exp of its own block, and
(b) "aux" matmul groups drained a few per block: the previous tile's
denominator-broadcast + O-projection, and the NEXT tile's Q^T/K^T/V projection
(projection runs one full tile ahead, so the first scores matmul of a new tile
issues immediately at the segment boundary and ScalarE never starves). This
keeps the tensor engine stream dense, which both hides the ~180ns per-matmul
ldweights/latency overhead and keeps the PE HAM warm.

Softmax denominators come free via an augmented ones-column in V (row 64 of
the PV accumulator). 1/l is computed with NO DMA: the l rows are copied
(cast bf16) to SBUF, broadcast to 64 partitions by a ones-vector matmul
(the ones row is borrowed from mask_sb[64, 64:128], which is all-ones by
construction), and a fast reciprocal reads the PSUM broadcast directly. The
whole release chain is SBUF/PSUM-local, so ReduceScatter HBM traffic can
never stall the inter-tile critical path through a DMA.

Output reduction: chunked ReduceScatter(add) over each 4-core batch group;
the last two tiles are processed in swapped order (7 then 6) so tile 7's RS
overlaps tile 6's attention and the final RS is a small single-tile chunk.
Everything crossing cores is bf16 (host casts final output to f32). RS-output
gathers are single DRAM->DRAM DMAs on the otherwise-idle GpSimd queue.

Startup: input DMA triggers are spread across the Sync/Scalar/Vector/GpSimd
queues (each trigger costs ~0.6us of queue time) so the first q/k projection
can begin ~5us earlier than with a single serial trigger queue. The ncfw
warm-up collective fires on GpSimd after that queue's x triggers (its trigger
blocks ~35us on the ncfw init barrier, so nothing else rides behind it).

B, S, D, H = 2, 4096, 512, 8 (hd=64). Hardcoded per problem spec.
"""

import numpy as np
import ml_dtypes

import concourse.bacc as bacc
import concourse.mybir as mybir
from concourse import tile
from concourse.bass_utils import run_bass_kernel_spmd

B, S, D = 2, 4096, 512
H = 8
HD = D // H          # 64
NCORES = 8
R = 128              # qkv rows per core (2 heads x 64)
S4 = S // 4          # output rows per core
NT = 8               # q tiles of 512
QW = 512             # q tile width
CHUNKS = [(0, 2), (2, 2), (4, 2), (7, 1), (6, 1)]   # (start tile, n tiles)
CHOFF = [0, 256, 512, 768, 896]                     # per-core out row offsets
# attention processing order: tile 7 BEFORE tile 6, so tile 7's
# ReduceScatter overlaps tile 6's attention and the final RS is the small
# tile-6 chunk right at the end of compute
ORDER = [0, 1, 2, 3, 4, 5, 7, 6]

BF16 = mybir.dt.bfloat16
F32 = mybir.dt.float32
AF = mybir.ActivationFunctionType
BF16_NP = ml_dtypes.bfloat16

_CACHE = {}


def _build_nc():
    nc = bacc.Bacc(num_devices=NCORES)

    # host pre-arranged so every weight DMA is contiguous
    xT = nc.declare_dram_parameter("xT", [D, S], BF16, isOutput=False)
    wqL = nc.declare_dram_parameter("wqL", [128, D], BF16, isOutput=False)
    wkL = nc.declare_dram_parameter("wkL", [128, D], BF16, isOutput=False)
    wvL = nc.declare_dram_parameter("wvL", [128, 4 * 130], BF16, isOutput=False)
    bq = nc.declare_dram_parameter("bq", [R, 1], F32, isOutput=False)
    bk = nc.declare_dram_parameter("bk", [R, 1], F32, isOutput=False)
    bvb = nc.declare_dram_parameter("bvb", [128, 130], F32, isOutput=False)
    wo0 = nc.declare_dram_parameter("wo0", [HD, D], BF16, isOutput=False)
    wo1 = nc.declare_dram_parameter("wo1", [HD, D], BF16, isOutput=False)
    bob4 = nc.declare_dram_parameter("bob4", [128, D], F32, isOutput=False)
    maskc = nc.declare_dram_parameter("maskc", [128, 4 * QW], BF16, isOutput=False)
    out_ext = nc.declare_dram_parameter("out", [S4, D], BF16, isOutput=True)

    parts = [nc.dram_tensor(f"part{ci}", [n * QW, D], BF16)
             for ci, (_, n) in enumerate(CHUNKS)]
    rss = [nc.dram_tensor(f"rs{ci}", [n * 128, D], BF16)
           for ci, (_, n) in enumerate(CHUNKS)]
    # tiny scratch RS to absorb the ~35us ncfw init barrier + first-op
    # latency during the idle early window, so the first real RS starts
    # promptly at its trigger
    cc_warm_in = nc.dram_tensor("ccwi", [4, 128], BF16)
    cc_warm_out = nc.dram_tensor("ccwo", [1, 128], BF16)

    tile2chunk = {}
    for ci, (st_, n) in enumerate(CHUNKS):
        for k in range(n):
            tile2chunk[st_ + k] = (ci, k, k == n - 1)

    with tile.TileContext(nc) as tc:
        with (
            tc.tile_pool(name="const", bufs=1) as cpool,
            tc.tile_pool(name="xres", bufs=1) as xpool,
            tc.tile_pool(name="pt", bufs=12) as ppool,
            tc.tile_pool(name="small", bufs=3) as spool,
            tc.tile_pool(name="stage", bufs=8) as stpool,
            tc.tile_pool(name="ps_s", bufs=2, space="PSUM") as ps_s,
            tc.tile_pool(name="ps_o", bufs=1, space="PSUM") as ps_o,
            tc.tile_pool(name="ps_pp", bufs=1, space="PSUM") as ps_pp,
            tc.tile_pool(name="ps_po", bufs=1, space="PSUM") as ps_po,
        ):
            # ---------- weights/constants + x, spread across DMA trigger
            # queues so the prologue q/k projection starts ASAP.
            bq_sb = cpool.tile([R, 1], F32)
            bk_sb = cpool.tile([R, 1], F32)
            wq_sb = cpool.tile([128, D], BF16)
            wk_sb = cpool.tile([128, D], BF16)
            nc.sync.dma_start(bq_sb[:], bq[:, :])
            nc.sync.dma_start(bk_sb[:], bk[:, :])
            nc.sync.dma_start(wq_sb[:], wqL[:, :])
            nc.sync.dma_start(wk_sb[:], wkL[:, :])

            xt = []
            for c in range(4):
                t_ = xpool.tile([128, S], BF16, tag=f"xt{c}")
                xt.append(t_)
            wv_sb = cpool.tile([128, 4 * 130], BF16)
            bvb_sb = cpool.tile([128, 130], F32)
            mask_sb = cpool.tile([128, 4 * QW], BF16)
            wo0_sb = cpool.tile([HD, D], BF16)
            wo1_sb = cpool.tile([HD, D], BF16)
            bob4_sb = cpool.tile([128, D], F32)

            # first 512-token x slice: spread across the three DMA-capable
            # trigger queues (Sync, Scalar, GpSimd)
            nc.scalar.dma_start(xt[0][:, 0:QW], xT[0:128, 0:QW])
            nc.scalar.dma_start(xt[1][:, 0:QW], xT[128:256, 0:QW])
            nc.gpsimd.dma_start(xt[2][:, 0:QW], xT[256:384, 0:QW])
            nc.gpsimd.dma_start(xt[3][:, 0:QW], xT[384:512, 0:QW])
            # fire the ncfw warm-up collective right after gpsimd's x triggers
            # (the trigger blocks ~35us on the ncfw init barrier, so nothing
            # else may ride behind it on the GpSimd queue)
            nc.gpsimd.collective_compute(
                "ReduceScatter",
                mybir.AluOpType.add,
                replica_groups=[[0, 1, 2, 3], [4, 5, 6, 7]],
                ins=[cc_warm_in[:, :]],
                outs=[cc_warm_out[:, :]],
            )
            # v/o weights + constants on the scalar queue (Scalar's first
            # ACTIVATE is ~10us in, so these triggers cost nothing)
            nc.scalar.dma_start(wv_sb[:], wvL[:, :])
            nc.scalar.dma_start(bvb_sb[:], bvb[:, :])
            nc.scalar.dma_start(mask_sb[:], maskc[:, :])
            nc.scalar.dma_start(wo0_sb[:], wo0[:, :])
            nc.scalar.dma_start(wo1_sb[:], wo1[:, :])
            nc.scalar.dma_start(bob4_sb[:], bob4[:, :])

            # remaining x: tile 1 first (projected during segment 0), then
            # the rest in big slices, all behind the weights on Sync
            for sl in ([slice(QW, 2 * QW)] +
                       [slice(q * (S // 4), (q + 1) * (S // 4)) for q in (1, 2, 3)]):
                for c in range(4):
                    nc.sync.dma_start(xt[c][:, sl], xT[128 * c:128 * (c + 1), sl])

            qT = xpool.tile([128, S], BF16, tag="qT")
            kT = xpool.tile([128, S], BF16, tag="kT")
            vaug = xpool.tile([128, 32 * 130], BF16, tag="vaug")

            # ---------- emission helpers (each is one "aux group")
            def g_qproj(pt):
                ps = ps_pp.tile([128, QW], F32, tag="pp")
                for c in range(4):
                    nc.tensor.matmul(
                        ps[:], wq_sb[:, 128 * c:128 * (c + 1)],
                        xt[c][:, QW * pt:QW * (pt + 1)],
                        start=(c == 0), stop=(c == 3),
                    )
                nc.vector.tensor_scalar_add(
                    qT[:, QW * pt:QW * (pt + 1)], ps[:], bq_sb[:])

            def g_kproj(pt):
                ps = ps_pp.tile([128, QW], F32, tag="pp")
                for c in range(4):
                    nc.tensor.matmul(
                        ps[:], wk_sb[:, 128 * c:128 * (c + 1)],
                        xt[c][:, QW * pt:QW * (pt + 1)],
                        start=(c == 0), stop=(c == 3),
                    )
                nc.vector.tensor_scalar_add(
                    kT[:, QW * pt:QW * (pt + 1)], ps[:], bk_sb[:])

            def g_vproj(tb):
                # tb: absolute 128-token block index (0..31)
                ps = ps_pp.tile([128, QW], F32, tag="pp")
                for c in range(4):
                    nc.tensor.matmul(
                        ps[:, 0:130],
                        xt[c][:, 128 * tb:128 * (tb + 1)],
                        wv_sb[:, 130 * c:130 * (c + 1)],
                        start=(c == 0), stop=(c == 3),
                    )
                nc.vector.tensor_add(
                    vaug[:, 130 * tb:130 * (tb + 1)], ps[:, 0:130], bvb_sb[:])

            rel = {}   # t -> dict of release-chain tiles

            def rel_start(t, o0, o1):
                # inline right after PV-stop: free the o banks + capture l rows
                oc0 = spool.tile([HD, QW], BF16, tag="oc0")
                oc1 = spool.tile([HD, QW], BF16, tag="oc1")
                nc.vector.tensor_copy(oc0[:], o0[0:64, :])
                nc.vector.tensor_copy(oc1[:], o1[0:64, :])
                lrow = spool.tile([128, 2 * QW], BF16, tag="lrow")
                nc.vector.tensor_copy(lrow[64:65, 0:QW], o0[64:65, :])
                nc.vector.tensor_copy(lrow[64:65, QW:2 * QW], o1[64:65, :])
                rel[t] = {"oc0": oc0, "oc1": oc1, "lrow": lrow}

            def g_linv(t):
                # broadcast l to 64 partitions via ones-vector matmul
                # (mask_sb[64, 64:128] is all-ones by construction), then
                # fast reciprocal straight off PSUM. No DMA anywhere.
                r = rel[t]
                linv = spool.tile([HD, 2 * QW], F32, tag="linv")
                for h in (0, 1):
                    lb = ps_po.tile([128, QW], F32, tag="po")
                    nc.tensor.matmul(
                        lb[0:64, :], mask_sb[64:65, 64:128],
                        r["lrow"][64:65, QW * h:QW * (h + 1)],
                        start=True, stop=True,
                    )
                    nc.vector.reciprocal_approx_fast(
                        linv[:, QW * h:QW * (h + 1)], lb[0:64, :])
                r["linv"] = linv

            def g_ocn(t):
                r = rel[t]
                ocn0 = spool.tile([HD, QW], BF16, tag="ocn0")
                ocn1 = spool.tile([HD, QW], BF16, tag="ocn1")
                nc.vector.tensor_mul(ocn0[:], r["oc0"], r["linv"][:, 0:QW])
                nc.vector.tensor_mul(ocn1[:], r["oc1"], r["linv"][:, QW:2 * QW])
                r["ocn0"] = ocn0
                r["ocn1"] = ocn1

            def g_oproj(t, tb):
                r = rel[t]
                ci, k, last_in_chunk = tile2chunk[t]
                po = ps_po.tile([128, QW], F32, tag="po")
                nc.tensor.matmul(po[:], r["ocn0"][:, 128 * tb:128 * (tb + 1)],
                                 wo0_sb[:], start=True, stop=False)
                nc.tensor.matmul(po[:], r["ocn1"][:, 128 * tb:128 * (tb + 1)],
                                 wo1_sb[:], start=False, stop=True)
                st = stpool.tile([128, QW], BF16, tag="st")
                nc.vector.tensor_add(st[:], po[:], bob4_sb[:])
                # parts writes ride the Sync queue, never GpSimd (whose first
                # collective trigger blocks on the ncfw init barrier)
                nc.sync.dma_start(
                    parts[ci][QW * k + 128 * tb:QW * k + 128 * (tb + 1), :],
                    st[:],
                )
                if tb == 3 and last_in_chunk:
                    nc.gpsimd.collective_compute(
                        "ReduceScatter",
                        mybir.AluOpType.add,
                        replica_groups=[[0, 1, 2, 3], [4, 5, 6, 7]],
                        ins=[parts[ci][:, :]],
                        outs=[rss[ci][:, :]],
                    )

            # ---------- prologue: project q/k for tile 0 only; v(0) and
            # everything else drains inside the segment loops
            g_qproj(0)
            g_kproj(0)

            proj_done = {0}
            prev_o = None            # (t, o0, o1) awaiting PV-stop release
            for i, t in enumerate(ORDER):
                nj = 4 * t + 4
                # -- aux groups for this segment
                aux = []
                if i > 0:
                    p = ORDER[i - 1]
                    aux.append(lambda p=p: g_linv(p))
                    aux.append(lambda p=p: g_ocn(p))
                if t == 0:
                    for b in range(4):
                        aux.append(lambda b=b: g_vproj(b))
                if i + 1 < len(ORDER):
                    nxt = ORDER[i + 1]
                    missing = [m for m in range(nxt + 1) if m not in proj_done]
                    for m in missing:
                        aux.append(lambda m=m: g_qproj(m))
                        aux.append(lambda m=m: g_kproj(m))
                    proj_done.update(missing)
                    for m in missing:
                        for b in range(4 * m, 4 * m + 4):
                            aux.append(lambda b=b: g_vproj(b))
                if i > 0:
                    p = ORDER[i - 1]
                    for tb in range(4):
                        aux.append(lambda p=p, tb=tb: g_oproj(p, tb))

                o0 = ps_o.tile([65, QW], F32, tag="o0")
                o1 = ps_o.tile([65, QW], F32, tag="o1")
                drained = 0

                def emit_pv(j):
                    q0 = max(0, 128 * (j - 4 * t))
                    pj = pv_pending_map[j]
                    for h, oo in ((0, o0), (1, o1)):
                        nc.tensor.matmul(
                            oo[:, q0:QW],
                            vaug[:, 130 * j + 65 * h:130 * j + 65 * (h + 1)],
                            pj[:, QW * h + q0:QW * (h + 1)],
                            start=(j == 0), stop=(j == nj - 1),
                        )

                pv_pending_map = {}
                for j in range(nj):
                    # scores for block j
                    q0 = max(0, 128 * (j - 4 * t))
                    s = ps_s.tile([128, 2 * QW], F32, tag="s")
                    for h in (0, 1):
                        nc.tensor.matmul(
                            s[:, QW * h + q0:QW * (h + 1)],
                            kT[64 * h:64 * (h + 1), 128 * j:128 * (j + 1)],
                            qT[64 * h:64 * (h + 1), QW * t + q0:QW * (t + 1)],
                            start=True, stop=True,
                        )
                    pj = ppool.tile([128, 2 * QW], BF16, tag="p")
                    if q0 == 0:
                        nc.scalar.activation(pj[:], s[:], AF.Exp, bias=0.0, scale=0.125)
                    else:
                        sv = s[:].rearrange("k (h q) -> k h q", h=2)[:, :, q0:QW]
                        pv = pj[:].rearrange("k (h q) -> k h q", h=2)[:, :, q0:QW]
                        nc.scalar.activation(pv, sv, AF.Exp, bias=0.0, scale=0.125)
                    if j >= 4 * t:  # diagonal 128-col boundary: 0/1 mask
                        for h in (0, 1):
                            nc.vector.tensor_mul(
                                pj[:, QW * h + q0:QW * h + q0 + 128],
                                pj[:, QW * h + q0:QW * h + q0 + 128],
                                mask_sb[:, 0:128],
                            )
                    pv_pending_map[j] = pj
                    # PV for the PREVIOUS block (its exp ran during this
                    # block's scores matmuls, so the PE never queue-blocks)
                    if j > 0:
                        emit_pv(j - 1)
                    # drain aux groups, spread evenly over the segment
                    want = (len(aux) * (j + 1) + nj - 1) // nj
                    while drained < want:
                        aux[drained]()
                        drained += 1
                emit_pv(nj - 1)
                rel_start(t, o0, o1)

            # final tile's release + O-projection + RS (nothing left to
            # overlap with -- keep the chain short)
            last = ORDER[-1]
            g_linv(last)
            g_ocn(last)
            for tb in range(4):
                g_oproj(last, tb)

            # ---- gather RS outputs: one DRAM->DRAM DMA per chunk on the
            # otherwise-idle GpSimd queue
            for ci, (st_, n) in enumerate(CHUNKS):
                nc.gpsimd.dma_start(
                    out_ext[CHOFF[ci]:CHOFF[ci] + n * 128, :], rss[ci][:, :])

    nc.finalize()
    return nc


def _make_in_maps(x, Wqkv, bqkv, Wo, bo):
    # causal 0/1 multiplicative masks for the 4 diagonal sub-block offsets:
    # keep (p, o) where o >= 128*r + p  (k = 128*(4t+r)+p, q = 512*t+o)
    # NOTE: row 64 of the r=0 block (cols 64:128) is all-ones; the kernel
    # borrows it as the ones-vector for the denominator broadcast matmul.
    p_idx = np.arange(128)[:, None]
    o_idx = np.arange(QW)[None, :]
    maskc = np.concatenate(
        [(o_idx >= 128 * r + p_idx).astype(np.float32) for r in range(4)], axis=1
    ).astype(BF16_NP)

    in_maps = []
    for core in range(NCORES):
        b = core // 4
        g = core % 4
        rows = slice(128 * g, 128 * (g + 1))
        wq = Wqkv[0:D][rows]            # [128, 512]
        wk = Wqkv[D:2 * D][rows]
        wv = Wqkv[2 * D:3 * D][rows]
        wvT = np.zeros((D, 130), dtype=np.float32)
        wvT[:, 0:64] = wv[0:64].T
        wvT[:, 65:129] = wv[64:128].T
        bvb = np.zeros((128, 130), dtype=np.float32)
        bvb[:, 0:64] = bqkv[2 * D:3 * D][rows][0:64][None, :]
        bvb[:, 64] = 1.0
        bvb[:, 65:129] = bqkv[2 * D:3 * D][rows][64:128][None, :]
        bvb[:, 129] = 1.0
        # device SBUF layouts, pre-arranged so DMAs are contiguous:
        # wq_sb[p, 128c+m] = wq.T[128c+p, m]
        wqT = np.ascontiguousarray(wq.T)        # [512, 128]
        wkT = np.ascontiguousarray(wk.T)
        wqL = wqT.reshape(4, 128, 128).transpose(1, 0, 2).reshape(128, 512)
        wkL = wkT.reshape(4, 128, 128).transpose(1, 0, 2).reshape(128, 512)
        wvL = wvT.reshape(4, 128, 130).transpose(1, 0, 2).reshape(128, 520)
        in_maps.append({
            "xT": np.ascontiguousarray(x[b].T).astype(BF16_NP),
            "wqL": np.ascontiguousarray(wqL).astype(BF16_NP),
            "wkL": np.ascontiguousarray(wkL).astype(BF16_NP),
            "wvL": np.ascontiguousarray(wvL).astype(BF16_NP),
            "bq": np.ascontiguousarray(bqkv[0:D][rows][:, None]).astype(np.float32),
            "bk": np.ascontiguousarray(bqkv[D:2 * D][rows][:, None]).astype(np.float32),
            "bvb": bvb,
            "wo0": np.ascontiguousarray(Wo[:, 128 * g:128 * g + 64].T).astype(BF16_NP),
            "wo1": np.ascontiguousarray(Wo[:, 128 * g + 64:128 * (g + 1)].T).astype(BF16_NP),
            "bob4": np.tile((bo / 4.0).astype(np.float32)[None, :], (128, 1)),
            "maskc": maskc,
        })
    return in_maps


def run(x, Wqkv, bqkv, Wo, bo, trace=False):
    if "nc" not in _CACHE:
        _CACHE["nc"] = _build_nc()
    nc = _CACHE["nc"]
    in_maps = _make_in_maps(x, Wqkv, bqkv, Wo, bo)
    res = run_bass_kernel_spmd(nc, in_maps, core_ids=list(range(NCORES)), trace=trace)
    out = np.empty((B, S, D), dtype=np.float32)
    for core in range(NCORES):
        b = core // 4
        r = core % 4
        o = np.asarray(res.results[core]["out"], dtype=np.float32)
        # chunk ci covers q rows [512*st, 512*(st+n)); rank r holds the r-th
        # quarter of the chunk, stored at out rows CHOFF[ci]..+n*128
        for ci, (st_, n) in enumerate(CHUNKS):
            sz = n * 128
            out[b, QW * st_ + sz * r:QW * st_ + sz * (r + 1), :] = \
                o[CHOFF[ci]:CHOFF[ci] + sz]
    return out, res


def kernel(x, Wqkv, bqkv, Wo, bo):
    out, _ = run(np.asarray(x, dtype=np.float32), np.asarray(Wqkv, dtype=np.float32),
                 np.asarray(bqkv, dtype=np.float32), np.asarray(Wo, dtype=np.float32),
                 np.asarray(bo, dtype=np.float32))
    return out
